# revision 11
# baseline (speedup 1.0000x reference)
"""Trainium2 Bass kernel for a non-selective (LTI) SSM.

Reference computation (per batch b, channel d):
    h_l = A @ h_{l-1} + Bvec * u[b, d, l]        (h in R^N, A = diag(a))
    y[b, d, l] = Cvec . h_l

Because the system is LTI with diagonal A, the scan collapses into a causal
convolution with taps k_j = sum_i C_i a_i^j B_i.  The taps decay as a_max^j,
so the convolution is effectively banded: we truncate it at NB*128 taps where
NB is the smallest block count whose dropped tail has relative L2 norm below
TAP_TAIL_TOL (NB=1..2 for typical uniform-spectrum A).

The banded convolution is computed as NB matmuls per chunk of 128 timesteps:

    y[c] = sum_b  T_b @ u[c-b],   T_b[t, k] = w_{128*b + t - k}

with chunks processed two at a time (free dim 512) so every matmul is a full
128x128x512 bf16 PE instruction.  No state, no scan, no recurrence.

Sharding: data-parallel over d_model (512 / 8 cores = 64 channels/core);
each core processes S = 4 batches x 64 channels = 256 sequences, with the
time axis laid out partition-major ([t_within_chunk, chunk, seq]) so every
DMA is fully contiguous per partition.  All HBM I/O is bf16 (the 2e-2
tolerance dwarfs bf16 rounding), halving DMA traffic vs fp32.
"""

import sys

sys.path.insert(0, "/opt/trn_rl_repo")

import numpy as np

import concourse.bass as bass
import concourse.mybir as mybir
import concourse.tile as tile
from concourse import bacc
from concourse.bass_utils import run_bass_kernel_spmd

N_CORES = 8
BATCH = 4
D_MODEL = 512
SEQ_LEN = 2048
N_STATE = 64
Q = 128                       # chunk length == partition dim
NCHUNK = SEQ_LEN // Q         # 16
NPAIR = NCHUNK // 2           # 8 (chunks are processed in pairs, N=512)
D_PER_CORE = D_MODEL // N_CORES  # 64
S = BATCH * D_PER_CORE        # 256 sequences per core
F32 = mybir.dt.float32
BF16 = mybir.dt.bfloat16
BF16_NP = mybir.dt.np(mybir.dt.bfloat16)
TAP_TAIL_TOL = 5e-3           # truncation budget (tolerance gate is 2e-2)

N_WARMUP = 10                 # dummy matmuls to lift the PE HAM clock gate
# Input transfers spread over the three DMA paths (sync/scalar HWDGE plus
# gpsimd SWDGE) — a single hardware queue only sustains ~160-190 GB/s, so
# queue-parallelism is required to reach the ~358 GB/s HBM limit.
IN_SYNC = [(0, 1), (1, 2), (6, 8)]   # pair ranges on the sync queue
IN_SCAL = [(2, 4)]                   # pair ranges on the scalar queue
IN_GPS = [(4, 6)]                    # pair ranges on the gpsimd queue
MM_GRP = 2                    # pairs per matmul phase (banks must alternate)


def build_program(nb):
    """Per-core Bass program (identical on all 8 cores) for nb tap blocks."""
    nc = bacc.Bacc(None, target_bir_lowering=False)

    pad = (nb - 1) * S        # zero columns for the left boundary
    ucols = pad + NCHUNK * S

    u_d = nc.declare_dram_parameter("u", [Q, NCHUNK * S], BF16, isOutput=False)
    cs_d = nc.declare_dram_parameter("consts", [Q, nb * Q], BF16, isOutput=False)
    y_d = nc.declare_dram_parameter("y", [Q, NCHUNK * S], BF16, isOutput=True)

    with tile.TileContext(nc) as tc:
        with (
            tc.tile_pool(name="consts", bufs=1) as cpool,
            tc.tile_pool(name="upool", bufs=1) as upool,
            tc.tile_pool(name="ypool", bufs=1) as ypool,
            tc.tile_pool(name="ps_w", bufs=1, space="PSUM") as ps_w,
            tc.tile_pool(name="ps_y", bufs=6, space="PSUM") as ps_y,
        ):
            # ---- weights via the scalar queue (keeps the sync queue free
            # for the earliest u transfers)
            cs = cpool.tile([Q, nb * Q], BF16)
            nc.scalar.dma_start(out=cs[:], in_=cs_d[:])

            # ---- input: single SBUF-resident tile, partition-major layout.
            # Left zero-pad supplies u[c-b] for the first chunks and doubles
            # as a zero operand for the PE warm-up matmuls below.
            ua = upool.tile([Q, ucols], BF16, name="ua", tag="ua")
            nc.vector.memset(ua[:, 0:pad], 0.0)

            def in_dma(eng, groups):
                for p0, p1 in groups:
                    eng.dma_start(
                        out=ua[:, pad + p0 * 2 * S: pad + p1 * 2 * S],
                        in_=u_d[:, p0 * 2 * S: p1 * 2 * S],
                    )
            in_dma(nc.sync, IN_SYNC)
            in_dma(nc.scalar, IN_SCAL)
            in_dma(nc.gpsimd, IN_GPS)

            # ---- PE warm-up on the zero pad: lifts the HAM clock gate
            # (1.2 -> 2.4 GHz) during the initial DMA wait without touching
            # input data.
            wps = ps_w.tile([Q, S], F32)
            for _ in range(N_WARMUP):
                nc.tensor.matmul(wps[:], ua[:, 0:Q], ua[:, 0:S],
                                 start=True, stop=True)

            ysb = ypool.tile([Q, NCHUNK * S], BF16, name="ysb", tag="ysb")

            # Matmul order [T0_j, T0_j+1, T1_j, T1_j+1]: consecutive MMs hit
            # different PSUM banks (same-bank back-to-back MMs stall ~200 ns)
            # and same-weight runs avoid the weight-swap penalty.  PSUM is
            # drained per pair with the two halves split across DVE and ACT.
            out_eng = [nc.sync, nc.scalar, nc.gpsimd, None]
            for g in range(NPAIR // MM_GRP):
                g0 = g * MM_GRP
                pys = [ps_y.tile([Q, 2 * S], F32, name="py", tag="py")
                       for _ in range(MM_GRP)]
                for b in range(nb):
                    for idx in range(MM_GRP):
                        j = g0 + idx
                        lo = pad + (2 * j - b) * S
                        nc.tensor.matmul(
                            pys[idx][:], cs[:, b * Q:(b + 1) * Q],
                            ua[:, lo: lo + 2 * S],
                            start=(b == 0), stop=(b == nb - 1),
                        )
                for idx in range(MM_GRP):
                    o = 2 * (g0 + idx) * S
                    nc.vector.tensor_copy(out=ysb[:, o: o + S],
                                          in_=pys[idx][:, 0: S])
                    nc.scalar.copy(out=ysb[:, o + S: o + 2 * S],
                                   in_=pys[idx][:, S: 2 * S])
                # flush the finished group; the last one splits across two
                # queues so the tail transfer halves.
                o0, o1 = 2 * g0 * S, 2 * (g0 + MM_GRP) * S
                if out_eng[g] is not None:
                    out_eng[g].dma_start(out=y_d[:, o0:o1],
                                         in_=ysb[:, o0:o1])
                else:
                    om = (o0 + o1) // 2
                    nc.sync.dma_start(out=y_d[:, o0:om], in_=ysb[:, o0:om])
                    nc.scalar.dma_start(out=y_d[:, om:o1], in_=ysb[:, om:o1])

    nc.compile()
    return nc


def make_params(A, Bvec, Cvec):
    """Host-side precompute: taps -> block-Toeplitz weights (lhsT layout)."""
    a = np.diag(np.asarray(A, np.float64))
    g = np.asarray(Bvec, np.float64) * np.asarray(Cvec, np.float64)
    t = np.arange(SEQ_LEN)
    w = (a[None, :] ** t[:, None]) @ g          # taps w[0..L-1]
    tail2 = np.cumsum((w ** 2)[::-1])[::-1]     # tail2[d] = sum_{j>=d} w_j^2
    total2 = tail2[0]
    # Output position t of a chunk sees taps d <= (nb-1)*Q + t, so the
    # dropped energy averaged over positions is mean_t tail2[(nb-1)*Q + t].
    nb = NCHUNK
    for k in range(1, NCHUNK):
        lo = (k - 1) * Q
        err2 = tail2[lo:lo + Q].mean() / max(total2, 1e-30)
        if np.sqrt(err2) <= TAP_TAIL_TOL:
            nb = k
            break
    # consts[k, b*Q + t] = T_b[t, k] = w_{b*Q + t - k}  (lhsT is transposed)
    tt, kk = np.meshgrid(np.arange(Q), np.arange(Q), indexing="ij")
    consts = np.zeros((Q, nb * Q), np.float64)
    for b in range(nb):
        d = b * Q + tt - kk                     # tap index per (t, k)
        m = np.where((d >= 0) & (d < SEQ_LEN), w[np.clip(d, 0, SEQ_LEN - 1)],
                     0.0)
        consts[:, b * Q:(b + 1) * Q] = m.T      # [k, t]
    return consts.astype(BF16_NP), nb


_prog_cache = {}


def get_program(nb):
    if nb not in _prog_cache:
        _prog_cache[nb] = build_program(nb)
    return _prog_cache[nb]


def shard_inputs(u, A, Bvec, Cvec):
    """FULL inputs -> (per-core in_maps, nb)."""
    consts, nb = make_params(A, Bvec, Cvec)
    u = np.asarray(u, np.float32)
    in_maps = []
    for core in range(N_CORES):
        us = u[:, core * D_PER_CORE:(core + 1) * D_PER_CORE, :]  # (B, Dc, L)
        us = us.reshape(S, SEQ_LEN).T.reshape(NCHUNK, Q, S)      # (c, t, s)
        us = np.ascontiguousarray(us.transpose(1, 0, 2))         # (t, c, s)
        in_maps.append({
            "u": us.reshape(Q, NCHUNK * S).astype(BF16_NP),
            "consts": consts,
        })
    return in_maps, nb


def unshard_output(results):
    """Per-core y shards -> FULL (B, D, L) fp32 output."""
    out = np.empty((BATCH, D_MODEL, SEQ_LEN), np.float32)
    for core in range(N_CORES):
        ys = np.asarray(results[core]["y"], np.float32)
        ys = ys.reshape(Q, NCHUNK, S).transpose(1, 0, 2)         # (c, t, s)
        ys = ys.reshape(SEQ_LEN, S).T                            # (S, L)
        out[:, core * D_PER_CORE:(core + 1) * D_PER_CORE, :] = ys.reshape(
            BATCH, D_PER_CORE, SEQ_LEN
        )
    return out


def kernel(u, A, Bvec, Cvec, L):
    u = np.asarray(u)
    assert u.shape == (BATCH, D_MODEL, SEQ_LEN), u.shape
    in_maps, nb = shard_inputs(u, A, Bvec, Cvec)
    nc = get_program(nb)
    res = run_bass_kernel_spmd(nc, in_maps, list(range(N_CORES)))
    return unshard_output(res.results)


# revision 14
# speedup vs baseline: 1.0919x; 1.0919x over previous
"""Trainium2 Bass kernel for a non-selective (LTI) SSM.

Reference computation (per batch b, channel d):
    h_l = A @ h_{l-1} + Bvec * u[b, d, l]        (h in R^N, A = diag(a))
    y[b, d, l] = Cvec . h_l

Because the system is LTI with diagonal A, the scan collapses into a causal
convolution with taps k_j = sum_i C_i a_i^j B_i.  The taps decay as a_max^j,
so the convolution is effectively banded: we truncate it at NB*128 taps where
NB is the smallest block count whose dropped tail has relative L2 norm below
TAP_TAIL_TOL (NB=1..2 for typical uniform-spectrum A).

The banded convolution is computed as NB matmuls per chunk of 128 timesteps:

    y[c] = sum_b  T_b @ u[c-b],   T_b[t, k] = w_{128*b + t - k}

with chunks processed two at a time (free dim 512) so every matmul is a full
128x128x512 bf16 PE instruction.  No state, no scan, no recurrence.

Sharding: data-parallel over d_model (512 / 8 cores = 64 channels/core);
each core processes S = 4 batches x 64 channels = 256 sequences, with the
time axis laid out partition-major ([t_within_chunk, chunk, seq]) so every
DMA is fully contiguous per partition.  All HBM I/O is bf16 (the 2e-2
tolerance dwarfs bf16 rounding), halving DMA traffic vs fp32.
"""

import sys

sys.path.insert(0, "/opt/trn_rl_repo")

import numpy as np

import concourse.bass as bass
import concourse.mybir as mybir
import concourse.tile as tile
from concourse import bacc
from concourse.bass_utils import run_bass_kernel_spmd

N_CORES = 8
BATCH = 4
D_MODEL = 512
SEQ_LEN = 2048
N_STATE = 64
Q = 128                       # chunk length == partition dim
NCHUNK = SEQ_LEN // Q         # 16
NPAIR = NCHUNK // 2           # 8 (chunks are processed in pairs, N=512)
D_PER_CORE = D_MODEL // N_CORES  # 64
S = BATCH * D_PER_CORE        # 256 sequences per core
F32 = mybir.dt.float32
BF16 = mybir.dt.bfloat16
BF16_NP = mybir.dt.np(mybir.dt.bfloat16)
TAP_TAIL_TOL = 5e-3           # truncation budget (tolerance gate is 2e-2)

N_WARMUP = 10                 # dummy matmuls to lift the PE HAM clock gate
# Input transfers across the two HWDGE queues (sync + scalar).  Measured:
# per-partition lines below 2 KB tank the per-queue rate (1 KB lines ->
# ~114 GB/s, 2 KB lines -> ~316 GB/s), and SWDGE (gpsimd) only adds
# contention — so every transfer is a 2-pair group (256 KB, 2 KB lines).
IN_SYNC = [(0, 2), (6, 8)]           # pair ranges on the sync queue
IN_SCAL = [(2, 4), (4, 6)]           # pair ranges on the scalar queue
MM_GRP = 2                    # pairs per matmul phase (banks must alternate)


def build_program(nb):
    """Per-core Bass program (identical on all 8 cores) for nb tap blocks."""
    nc = bacc.Bacc(None, target_bir_lowering=False)

    pad = (nb - 1) * S        # zero columns for the left boundary
    ucols = pad + NCHUNK * S

    u_d = nc.declare_dram_parameter("u", [Q, NCHUNK * S], BF16, isOutput=False)
    cs_d = nc.declare_dram_parameter("consts", [Q, nb * Q], BF16, isOutput=False)
    y_d = nc.declare_dram_parameter("y", [Q, NCHUNK * S], BF16, isOutput=True)

    with tile.TileContext(nc) as tc:
        with (
            tc.tile_pool(name="consts", bufs=1) as cpool,
            tc.tile_pool(name="upool", bufs=1) as upool,
            tc.tile_pool(name="ypool", bufs=1) as ypool,
            tc.tile_pool(name="ps_w", bufs=1, space="PSUM") as ps_w,
            tc.tile_pool(name="ps_y", bufs=6, space="PSUM") as ps_y,
        ):
            # ---- weights ride first on the sync queue (tiny transfer)
            cs = cpool.tile([Q, nb * Q], BF16)
            nc.sync.dma_start(out=cs[:], in_=cs_d[:])

            # ---- input: single SBUF-resident tile, partition-major layout.
            # Left zero-pad supplies u[c-b] for the first chunks and doubles
            # as a zero operand for the PE warm-up matmuls below.
            ua = upool.tile([Q, ucols], BF16, name="ua", tag="ua")
            nc.vector.memset(ua[:, 0:pad], 0.0)

            def in_dma(eng, groups):
                for p0, p1 in groups:
                    eng.dma_start(
                        out=ua[:, pad + p0 * 2 * S: pad + p1 * 2 * S],
                        in_=u_d[:, p0 * 2 * S: p1 * 2 * S],
                    )
            in_dma(nc.sync, IN_SYNC)
            in_dma(nc.scalar, IN_SCAL)

            # ---- PE warm-up on the zero pad: lifts the HAM clock gate
            # (1.2 -> 2.4 GHz) during the initial DMA wait without touching
            # input data.
            wps = ps_w.tile([Q, S], F32)
            for _ in range(N_WARMUP):
                nc.tensor.matmul(wps[:], ua[:, 0:Q], ua[:, 0:S],
                                 start=True, stop=True)

            ysb = ypool.tile([Q, NCHUNK * S], BF16, name="ysb", tag="ysb")

            # Matmul order [T0_j, T0_j+1, T1_j, T1_j+1]: consecutive MMs hit
            # different PSUM banks (same-bank back-to-back MMs stall ~200 ns)
            # and same-weight runs avoid the weight-swap penalty.  PSUM is
            # drained per pair with the two halves split across DVE and ACT.
            out_eng = [nc.scalar, nc.sync, nc.scalar, None]
            for g in range(NPAIR // MM_GRP):
                g0 = g * MM_GRP
                pys = [ps_y.tile([Q, 2 * S], F32, name="py", tag="py")
                       for _ in range(MM_GRP)]
                for b in range(nb):
                    for idx in range(MM_GRP):
                        j = g0 + idx
                        lo = pad + (2 * j - b) * S
                        nc.tensor.matmul(
                            pys[idx][:], cs[:, b * Q:(b + 1) * Q],
                            ua[:, lo: lo + 2 * S],
                            start=(b == 0), stop=(b == nb - 1),
                        )
                for idx in range(MM_GRP):
                    o = 2 * (g0 + idx) * S
                    nc.vector.tensor_copy(out=ysb[:, o: o + S],
                                          in_=pys[idx][:, 0: S])
                    nc.scalar.copy(out=ysb[:, o + S: o + 2 * S],
                                   in_=pys[idx][:, S: 2 * S])
                # flush the finished group; the last one splits across two
                # queues so the tail transfer halves.
                o0, o1 = 2 * g0 * S, 2 * (g0 + MM_GRP) * S
                if out_eng[g] is not None:
                    out_eng[g].dma_start(out=y_d[:, o0:o1],
                                         in_=ysb[:, o0:o1])
                else:
                    om = (o0 + o1) // 2
                    nc.sync.dma_start(out=y_d[:, o0:om], in_=ysb[:, o0:om])
                    nc.scalar.dma_start(out=y_d[:, om:o1], in_=ysb[:, om:o1])

    nc.compile()
    return nc


def make_params(A, Bvec, Cvec):
    """Host-side precompute: taps -> block-Toeplitz weights (lhsT layout)."""
    a = np.diag(np.asarray(A, np.float64))
    g = np.asarray(Bvec, np.float64) * np.asarray(Cvec, np.float64)
    t = np.arange(SEQ_LEN)
    w = (a[None, :] ** t[:, None]) @ g          # taps w[0..L-1]
    tail2 = np.cumsum((w ** 2)[::-1])[::-1]     # tail2[d] = sum_{j>=d} w_j^2
    total2 = tail2[0]
    # Output position t of a chunk sees taps d <= (nb-1)*Q + t, so the
    # dropped energy averaged over positions is mean_t tail2[(nb-1)*Q + t].
    nb = NCHUNK
    for k in range(1, NCHUNK):
        lo = (k - 1) * Q
        err2 = tail2[lo:lo + Q].mean() / max(total2, 1e-30)
        if np.sqrt(err2) <= TAP_TAIL_TOL:
            nb = k
            break
    # consts[k, b*Q + t] = T_b[t, k] = w_{b*Q + t - k}  (lhsT is transposed)
    tt, kk = np.meshgrid(np.arange(Q), np.arange(Q), indexing="ij")
    consts = np.zeros((Q, nb * Q), np.float64)
    for b in range(nb):
        d = b * Q + tt - kk                     # tap index per (t, k)
        m = np.where((d >= 0) & (d < SEQ_LEN), w[np.clip(d, 0, SEQ_LEN - 1)],
                     0.0)
        consts[:, b * Q:(b + 1) * Q] = m.T      # [k, t]
    return consts.astype(BF16_NP), nb


_prog_cache = {}


def get_program(nb):
    if nb not in _prog_cache:
        _prog_cache[nb] = build_program(nb)
    return _prog_cache[nb]


def shard_inputs(u, A, Bvec, Cvec):
    """FULL inputs -> (per-core in_maps, nb)."""
    consts, nb = make_params(A, Bvec, Cvec)
    u = np.asarray(u, np.float32)
    in_maps = []
    for core in range(N_CORES):
        us = u[:, core * D_PER_CORE:(core + 1) * D_PER_CORE, :]  # (B, Dc, L)
        us = us.reshape(S, SEQ_LEN).T.reshape(NCHUNK, Q, S)      # (c, t, s)
        us = np.ascontiguousarray(us.transpose(1, 0, 2))         # (t, c, s)
        in_maps.append({
            "u": us.reshape(Q, NCHUNK * S).astype(BF16_NP),
            "consts": consts,
        })
    return in_maps, nb


def unshard_output(results):
    """Per-core y shards -> FULL (B, D, L) fp32 output."""
    out = np.empty((BATCH, D_MODEL, SEQ_LEN), np.float32)
    for core in range(N_CORES):
        ys = np.asarray(results[core]["y"], np.float32)
        ys = ys.reshape(Q, NCHUNK, S).transpose(1, 0, 2)         # (c, t, s)
        ys = ys.reshape(SEQ_LEN, S).T                            # (S, L)
        out[:, core * D_PER_CORE:(core + 1) * D_PER_CORE, :] = ys.reshape(
            BATCH, D_PER_CORE, SEQ_LEN
        )
    return out


def kernel(u, A, Bvec, Cvec, L):
    u = np.asarray(u)
    assert u.shape == (BATCH, D_MODEL, SEQ_LEN), u.shape
    in_maps, nb = shard_inputs(u, A, Bvec, Cvec)
    nc = get_program(nb)
    res = run_bass_kernel_spmd(nc, in_maps, list(range(N_CORES)))
    return unshard_output(res.results)


# revision 17
# speedup vs baseline: 1.1204x; 1.0261x over previous
"""Trainium2 Bass kernel for a non-selective (LTI) SSM.

Reference computation (per batch b, channel d):
    h_l = A @ h_{l-1} + Bvec * u[b, d, l]        (h in R^N, A = diag(a))
    y[b, d, l] = Cvec . h_l

Because the system is LTI with diagonal A, the scan collapses into a causal
convolution with taps k_j = sum_i C_i a_i^j B_i.  The taps decay as a_max^j,
so the convolution is effectively banded: we truncate it at NB*128 taps where
NB is the smallest block count whose dropped tail has relative L2 norm below
TAP_TAIL_TOL (NB=1..2 for typical uniform-spectrum A).

The banded convolution is computed as NB matmuls per chunk of 128 timesteps:

    y[c] = sum_b  T_b @ u[c-b],   T_b[t, k] = w_{128*b + t - k}

with chunks processed two at a time (free dim 512) so every matmul is a full
128x128x512 bf16 PE instruction.  No state, no scan, no recurrence.

Sharding: data-parallel over d_model (512 / 8 cores = 64 channels/core);
each core processes S = 4 batches x 64 channels = 256 sequences, with the
time axis laid out partition-major ([t_within_chunk, chunk, seq]) so every
DMA is fully contiguous per partition.  All HBM I/O is bf16 (the 2e-2
tolerance dwarfs bf16 rounding), halving DMA traffic vs fp32.
"""

import sys

sys.path.insert(0, "/opt/trn_rl_repo")

import numpy as np

import concourse.bass as bass
import concourse.mybir as mybir
import concourse.tile as tile
from concourse import bacc
from concourse.bass_utils import run_bass_kernel_spmd

N_CORES = 8
BATCH = 4
D_MODEL = 512
SEQ_LEN = 2048
N_STATE = 64
Q = 128                       # chunk length == partition dim
NCHUNK = SEQ_LEN // Q         # 16
NPAIR = NCHUNK // 2           # 8 (chunks are processed in pairs, N=512)
D_PER_CORE = D_MODEL // N_CORES  # 64
S = BATCH * D_PER_CORE        # 256 sequences per core
F32 = mybir.dt.float32
BF16 = mybir.dt.bfloat16
BF16_NP = mybir.dt.np(mybir.dt.bfloat16)
TAP_TAIL_TOL = 5e-3           # truncation budget (tolerance gate is 2e-2)

N_WARMUP = 14                 # dummy matmuls to lift the PE HAM clock gate;
                              # must span ~3.4us of CONTINUOUS PE activity
                              # bridging until the first input data lands
# Input transfers across the two HWDGE queues (sync + scalar).  Measured:
# per-partition lines below 2 KB tank the per-queue rate (1 KB lines ->
# ~114 GB/s, 2 KB lines -> ~316 GB/s), and SWDGE (gpsimd) only adds
# contention — so every transfer is a 2-pair group (256 KB, 2 KB lines).
IN_SYNC = [(0, 2), (6, 8)]           # pair ranges on the sync queue
IN_SCAL = [(2, 4), (4, 6)]           # pair ranges on the scalar queue
MM_GRP = 2                    # pairs per matmul phase (banks must alternate)


def build_program(nb):
    """Per-core Bass program (identical on all 8 cores) for nb tap blocks."""
    nc = bacc.Bacc(None, target_bir_lowering=False)

    pad = (nb - 1) * S        # zero columns for the left boundary
    ucols = pad + NCHUNK * S

    u_d = nc.declare_dram_parameter("u", [Q, NCHUNK * S], BF16, isOutput=False)
    cs_d = nc.declare_dram_parameter("consts", [Q, nb * Q], BF16, isOutput=False)
    y_d = nc.declare_dram_parameter("y", [Q, NCHUNK * S], BF16, isOutput=True)

    with tile.TileContext(nc) as tc:
        with (
            tc.tile_pool(name="consts", bufs=1) as cpool,
            tc.tile_pool(name="upool", bufs=1) as upool,
            tc.tile_pool(name="ypool", bufs=1) as ypool,
            tc.tile_pool(name="ps_w", bufs=1, space="PSUM") as ps_w,
            tc.tile_pool(name="ps_y", bufs=6, space="PSUM") as ps_y,
        ):
            # ---- weights ride first on the scalar queue (tiny transfer)
            cs = cpool.tile([Q, nb * Q], BF16)
            nc.scalar.dma_start(out=cs[:], in_=cs_d[:])

            # ---- input: single SBUF-resident tile, partition-major layout.
            # Left zero-pad supplies u[c-b] for the first chunks and doubles
            # as a zero operand for the PE warm-up matmuls below.
            ua = upool.tile([Q, ucols], BF16, name="ua", tag="ua")
            nc.vector.memset(ua[:, 0:pad], 0.0)

            def in_dma(eng, groups):
                for p0, p1 in groups:
                    eng.dma_start(
                        out=ua[:, pad + p0 * 2 * S: pad + p1 * 2 * S],
                        in_=u_d[:, p0 * 2 * S: p1 * 2 * S],
                    )
            in_dma(nc.sync, IN_SYNC)
            in_dma(nc.scalar, IN_SCAL)

            # ---- PE warm-up on the zero pad: lifts the HAM clock gate
            # (1.2 -> 2.4 GHz) during the initial DMA wait without touching
            # input data.
            wps = ps_w.tile([Q, S], F32)
            for _ in range(N_WARMUP):
                nc.tensor.matmul(wps[:], ua[:, 0:Q], ua[:, 0:S],
                                 start=True, stop=True)

            ysb = ypool.tile([Q, NCHUNK * S], BF16, name="ysb", tag="ysb")

            # Matmul order [T0_j, T0_j+1, T1_j, T1_j+1]: consecutive MMs hit
            # different PSUM banks (same-bank back-to-back MMs stall ~200 ns)
            # and same-weight runs avoid the weight-swap penalty.  PSUM is
            # drained per pair with the two halves split across DVE and ACT.
            out_eng = [nc.sync, nc.scalar, nc.sync, None]
            for g in range(NPAIR // MM_GRP):
                g0 = g * MM_GRP
                pys = [ps_y.tile([Q, 2 * S], F32, name="py", tag="py")
                       for _ in range(MM_GRP)]
                for b in range(nb):
                    for idx in range(MM_GRP):
                        j = g0 + idx
                        lo = pad + (2 * j - b) * S
                        nc.tensor.matmul(
                            pys[idx][:], cs[:, b * Q:(b + 1) * Q],
                            ua[:, lo: lo + 2 * S],
                            start=(b == 0), stop=(b == nb - 1),
                        )
                for idx in range(MM_GRP):
                    o = 2 * (g0 + idx) * S
                    nc.vector.tensor_copy(out=ysb[:, o: o + S],
                                          in_=pys[idx][:, 0: S])
                    nc.scalar.copy(out=ysb[:, o + S: o + 2 * S],
                                   in_=pys[idx][:, S: 2 * S])
                # flush the finished group; the last one splits across two
                # queues so the tail transfer halves.
                o0, o1 = 2 * g0 * S, 2 * (g0 + MM_GRP) * S
                if out_eng[g] is not None:
                    out_eng[g].dma_start(out=y_d[:, o0:o1],
                                         in_=ysb[:, o0:o1])
                else:
                    om = (o0 + o1) // 2
                    nc.sync.dma_start(out=y_d[:, o0:om], in_=ysb[:, o0:om])
                    nc.scalar.dma_start(out=y_d[:, om:o1], in_=ysb[:, om:o1])

    nc.compile()
    return nc


def make_params(A, Bvec, Cvec):
    """Host-side precompute: taps -> block-Toeplitz weights (lhsT layout)."""
    a = np.diag(np.asarray(A, np.float64))
    g = np.asarray(Bvec, np.float64) * np.asarray(Cvec, np.float64)
    t = np.arange(SEQ_LEN)
    w = (a[None, :] ** t[:, None]) @ g          # taps w[0..L-1]
    tail2 = np.cumsum((w ** 2)[::-1])[::-1]     # tail2[d] = sum_{j>=d} w_j^2
    total2 = tail2[0]
    # Output position t of a chunk sees taps d <= (nb-1)*Q + t, so the
    # dropped energy averaged over positions is mean_t tail2[(nb-1)*Q + t].
    nb = NCHUNK
    for k in range(1, NCHUNK):
        lo = (k - 1) * Q
        err2 = tail2[lo:lo + Q].mean() / max(total2, 1e-30)
        if np.sqrt(err2) <= TAP_TAIL_TOL:
            nb = k
            break
    # consts[k, b*Q + t] = T_b[t, k] = w_{b*Q + t - k}  (lhsT is transposed)
    tt, kk = np.meshgrid(np.arange(Q), np.arange(Q), indexing="ij")
    consts = np.zeros((Q, nb * Q), np.float64)
    for b in range(nb):
        d = b * Q + tt - kk                     # tap index per (t, k)
        m = np.where((d >= 0) & (d < SEQ_LEN), w[np.clip(d, 0, SEQ_LEN - 1)],
                     0.0)
        consts[:, b * Q:(b + 1) * Q] = m.T      # [k, t]
    return consts.astype(BF16_NP), nb


_prog_cache = {}


def get_program(nb):
    if nb not in _prog_cache:
        _prog_cache[nb] = build_program(nb)
    return _prog_cache[nb]


def shard_inputs(u, A, Bvec, Cvec):
    """FULL inputs -> (per-core in_maps, nb)."""
    consts, nb = make_params(A, Bvec, Cvec)
    u = np.asarray(u, np.float32)
    in_maps = []
    for core in range(N_CORES):
        us = u[:, core * D_PER_CORE:(core + 1) * D_PER_CORE, :]  # (B, Dc, L)
        us = us.reshape(S, SEQ_LEN).T.reshape(NCHUNK, Q, S)      # (c, t, s)
        us = np.ascontiguousarray(us.transpose(1, 0, 2))         # (t, c, s)
        in_maps.append({
            "u": us.reshape(Q, NCHUNK * S).astype(BF16_NP),
            "consts": consts,
        })
    return in_maps, nb


def unshard_output(results):
    """Per-core y shards -> FULL (B, D, L) fp32 output."""
    out = np.empty((BATCH, D_MODEL, SEQ_LEN), np.float32)
    for core in range(N_CORES):
        ys = np.asarray(results[core]["y"], np.float32)
        ys = ys.reshape(Q, NCHUNK, S).transpose(1, 0, 2)         # (c, t, s)
        ys = ys.reshape(SEQ_LEN, S).T                            # (S, L)
        out[:, core * D_PER_CORE:(core + 1) * D_PER_CORE, :] = ys.reshape(
            BATCH, D_PER_CORE, SEQ_LEN
        )
    return out


def kernel(u, A, Bvec, Cvec, L):
    u = np.asarray(u)
    assert u.shape == (BATCH, D_MODEL, SEQ_LEN), u.shape
    in_maps, nb = shard_inputs(u, A, Bvec, Cvec)
    nc = get_program(nb)
    res = run_bass_kernel_spmd(nc, in_maps, list(range(N_CORES)))
    return unshard_output(res.results)


# revision 21
# speedup vs baseline: 1.1782x; 1.0516x over previous
"""Trainium2 Bass kernel for a non-selective (LTI) SSM.

Reference computation (per batch b, channel d):
    h_l = A @ h_{l-1} + Bvec * u[b, d, l]        (h in R^N, A = diag(a))
    y[b, d, l] = Cvec . h_l

Because the system is LTI with diagonal A, the scan collapses into a causal
convolution with taps k_j = sum_i C_i a_i^j B_i.  The taps decay as a_max^j,
so the convolution is effectively banded: we truncate it at NB*128 taps where
NB is the smallest block count whose dropped tail has relative L2 norm below
TAP_TAIL_TOL (NB=1..2 for typical uniform-spectrum A).

The banded convolution is computed as NB matmuls per chunk of 128 timesteps:

    y[c] = sum_b  T_b @ u[c-b],   T_b[t, k] = w_{128*b + t - k}

with chunks processed two at a time (free dim 512) so every matmul is a full
128x128x512 bf16 PE instruction.  No state, no scan, no recurrence.

Sharding: data-parallel over d_model (512 / 8 cores = 64 channels/core);
each core processes S = 4 batches x 64 channels = 256 sequences, with the
time axis laid out partition-major ([t_within_chunk, chunk, seq]) so every
DMA is fully contiguous per partition.  All HBM I/O is bf16 (the 2e-2
tolerance dwarfs bf16 rounding), halving DMA traffic vs fp32.
"""

import sys

sys.path.insert(0, "/opt/trn_rl_repo")

import numpy as np

import concourse.bass as bass
import concourse.mybir as mybir
import concourse.tile as tile
from concourse import bacc
from concourse.bass_utils import run_bass_kernel_spmd

N_CORES = 8
BATCH = 4
D_MODEL = 512
SEQ_LEN = 2048
N_STATE = 64
Q = 128                       # chunk length == partition dim
NCHUNK = SEQ_LEN // Q         # 16
NPAIR = NCHUNK // 2           # 8 (chunks are processed in pairs, N=512)
D_PER_CORE = D_MODEL // N_CORES  # 64
S = BATCH * D_PER_CORE        # 256 sequences per core
F32 = mybir.dt.float32
BF16 = mybir.dt.bfloat16
BF16_NP = mybir.dt.np(mybir.dt.bfloat16)
TAP_TAIL_TOL = 5e-3           # truncation budget (tolerance gate is 2e-2)

N_WARMUP = 30                 # dummy matmuls to lift the PE HAM clock gate;
                              # must span ~3.4us of CONTINUOUS PE activity
                              # bridging until the first input data lands —
                              # any PE idle gap resets the HAM ramp.  N=128
                              # keeps each one short (~107ns cold) so the
                              # bridge overshoot is small.
# Input transfers across the two HWDGE queues (sync + scalar).  Measured:
# per-partition lines below 2 KB tank the per-queue rate (1 KB lines ->
# ~114 GB/s, 2 KB lines -> ~316 GB/s), and SWDGE (gpsimd) only adds
# contention — so every transfer is a 2-pair group (256 KB, 2 KB lines).
IN_SYNC = [(0, 2), (6, 8)]           # pair ranges on the sync queue
IN_SCAL = [(2, 4), (4, 6)]           # pair ranges on the scalar queue
MM_GRP = 2                    # pairs per matmul phase (banks must alternate)


def build_program(nb):
    """Per-core Bass program (identical on all 8 cores) for nb tap blocks."""
    nc = bacc.Bacc(None, target_bir_lowering=False)

    pad = (nb - 1) * S        # zero columns for the left boundary
    ucols = pad + NCHUNK * S

    u_d = nc.declare_dram_parameter("u", [Q, NCHUNK * S], BF16, isOutput=False)
    cs_d = nc.declare_dram_parameter("consts", [Q, nb * Q], BF16, isOutput=False)
    y_d = nc.declare_dram_parameter("y", [Q, NCHUNK * S], BF16, isOutput=True)

    with tile.TileContext(nc) as tc:
        with (
            tc.tile_pool(name="consts", bufs=1) as cpool,
            tc.tile_pool(name="upool", bufs=1) as upool,
            tc.tile_pool(name="ypool", bufs=1) as ypool,
            tc.tile_pool(name="ps_w", bufs=1, space="PSUM") as ps_w,
            tc.tile_pool(name="ps_y", bufs=6, space="PSUM") as ps_y,
        ):
            # ---- weights ride first on the scalar queue (tiny transfer)
            cs = cpool.tile([Q, nb * Q], BF16)
            nc.scalar.dma_start(out=cs[:], in_=cs_d[:])

            # ---- input: single SBUF-resident tile, partition-major layout.
            # Left zero-pad supplies u[c-b] for the first chunks and doubles
            # as a zero operand for the PE warm-up matmuls below.
            ua = upool.tile([Q, ucols], BF16, name="ua", tag="ua")
            nc.vector.memset(ua[:, 0:pad], 0.0)

            def in_dma(eng, groups):
                for p0, p1 in groups:
                    eng.dma_start(
                        out=ua[:, pad + p0 * 2 * S: pad + p1 * 2 * S],
                        in_=u_d[:, p0 * 2 * S: p1 * 2 * S],
                    )
            in_dma(nc.sync, IN_SYNC)
            in_dma(nc.scalar, IN_SCAL)

            # ---- PE warm-up on the zero pad: lifts the HAM clock gate
            # (1.2 -> 2.4 GHz) during the initial DMA wait without touching
            # input data.
            wps = ps_w.tile([Q, S], F32)
            for _ in range(N_WARMUP):
                nc.tensor.matmul(wps[:, 0:Q], ua[:, 0:Q], ua[:, 0:Q],
                                 start=True, stop=True)

            ysb = ypool.tile([Q, NCHUNK * S], BF16, name="ysb", tag="ysb")

            # Matmul order [T0_j, T0_j+1, T1_j, T1_j+1]: consecutive MMs hit
            # different PSUM banks (same-bank back-to-back MMs stall ~200 ns)
            # and same-weight runs avoid the weight-swap penalty.  PSUM is
            # drained per pair with the two halves split across DVE and ACT.
            out_eng = [nc.sync, nc.sync, nc.sync, nc.sync]
            for g in range(NPAIR // MM_GRP):
                g0 = g * MM_GRP
                pys = [ps_y.tile([Q, 2 * S], F32, name="py", tag="py")
                       for _ in range(MM_GRP)]
                for b in range(nb):
                    for idx in range(MM_GRP):
                        j = g0 + idx
                        lo = pad + (2 * j - b) * S
                        nc.tensor.matmul(
                            pys[idx][:], cs[:, b * Q:(b + 1) * Q],
                            ua[:, lo: lo + 2 * S],
                            start=(b == 0), stop=(b == nb - 1),
                        )
                for idx in range(MM_GRP):
                    o = 2 * (g0 + idx) * S
                    nc.vector.tensor_copy(out=ysb[:, o: o + S],
                                          in_=pys[idx][:, 0: S])
                    nc.scalar.copy(out=ysb[:, o + S: o + 2 * S],
                                   in_=pys[idx][:, S: 2 * S])
                # flush the finished group.  All output issues ride the sync
                # queue: it is idle after the input issues, and an engine
                # that also runs copies would head-of-line-block its later
                # copies while waiting for the flush semaphore.
                o0, o1 = 2 * g0 * S, 2 * (g0 + MM_GRP) * S
                out_eng[g].dma_start(out=y_d[:, o0:o1], in_=ysb[:, o0:o1])

    nc.compile()
    return nc


def make_params(A, Bvec, Cvec):
    """Host-side precompute: taps -> block-Toeplitz weights (lhsT layout)."""
    a = np.diag(np.asarray(A, np.float64))
    g = np.asarray(Bvec, np.float64) * np.asarray(Cvec, np.float64)
    t = np.arange(SEQ_LEN)
    w = (a[None, :] ** t[:, None]) @ g          # taps w[0..L-1]
    tail2 = np.cumsum((w ** 2)[::-1])[::-1]     # tail2[d] = sum_{j>=d} w_j^2
    total2 = tail2[0]
    # Output position t of a chunk sees taps d <= (nb-1)*Q + t, so the
    # dropped energy averaged over positions is mean_t tail2[(nb-1)*Q + t].
    nb = NCHUNK
    for k in range(1, NCHUNK):
        lo = (k - 1) * Q
        err2 = tail2[lo:lo + Q].mean() / max(total2, 1e-30)
        if np.sqrt(err2) <= TAP_TAIL_TOL:
            nb = k
            break
    # consts[k, b*Q + t] = T_b[t, k] = w_{b*Q + t - k}  (lhsT is transposed)
    tt, kk = np.meshgrid(np.arange(Q), np.arange(Q), indexing="ij")
    consts = np.zeros((Q, nb * Q), np.float64)
    for b in range(nb):
        d = b * Q + tt - kk                     # tap index per (t, k)
        m = np.where((d >= 0) & (d < SEQ_LEN), w[np.clip(d, 0, SEQ_LEN - 1)],
                     0.0)
        consts[:, b * Q:(b + 1) * Q] = m.T      # [k, t]
    return consts.astype(BF16_NP), nb


_prog_cache = {}


def get_program(nb):
    if nb not in _prog_cache:
        _prog_cache[nb] = build_program(nb)
    return _prog_cache[nb]


def shard_inputs(u, A, Bvec, Cvec):
    """FULL inputs -> (per-core in_maps, nb)."""
    consts, nb = make_params(A, Bvec, Cvec)
    u = np.asarray(u, np.float32)
    in_maps = []
    for core in range(N_CORES):
        us = u[:, core * D_PER_CORE:(core + 1) * D_PER_CORE, :]  # (B, Dc, L)
        us = us.reshape(S, SEQ_LEN).T.reshape(NCHUNK, Q, S)      # (c, t, s)
        us = np.ascontiguousarray(us.transpose(1, 0, 2))         # (t, c, s)
        in_maps.append({
            "u": us.reshape(Q, NCHUNK * S).astype(BF16_NP),
            "consts": consts,
        })
    return in_maps, nb


def unshard_output(results):
    """Per-core y shards -> FULL (B, D, L) fp32 output."""
    out = np.empty((BATCH, D_MODEL, SEQ_LEN), np.float32)
    for core in range(N_CORES):
        ys = np.asarray(results[core]["y"], np.float32)
        ys = ys.reshape(Q, NCHUNK, S).transpose(1, 0, 2)         # (c, t, s)
        ys = ys.reshape(SEQ_LEN, S).T                            # (S, L)
        out[:, core * D_PER_CORE:(core + 1) * D_PER_CORE, :] = ys.reshape(
            BATCH, D_PER_CORE, SEQ_LEN
        )
    return out


def kernel(u, A, Bvec, Cvec, L):
    u = np.asarray(u)
    assert u.shape == (BATCH, D_MODEL, SEQ_LEN), u.shape
    in_maps, nb = shard_inputs(u, A, Bvec, Cvec)
    nc = get_program(nb)
    res = run_bass_kernel_spmd(nc, in_maps, list(range(N_CORES)))
    return unshard_output(res.results)
